# revision 6
# baseline (speedup 1.0000x reference)
# Trainium2 Bass kernel for nn_EquivariantLocalScoreMachine (retrieval_knn).
#
# Math: for each spatial site s=(b,y,x) (S=2048) and dataset patch p (P=32768):
#   w[p,s] = (mu*conv[p,s] - (x_norms[s] + mu^2*pnorms[p])/2) / sigma^2
#   out[c,s] = (mu * sum_p e^w*pcent[p,c] / sum_p e^w - x[c,s]) / sigma^2
# The output is invariant to any per-site offset of w, so instead of an exact
# online max we fold a cheap Cauchy-Schwarz upper bound M~[s] (computed on the
# host) into the matmul and do a single pass.
#
# Device kernel (per core, patches sharded 8 ways -> 4096 patches/core):
#   - w-matmul: lhsT=[87,128] patch tile, rhs=[87,512] x-side tile -> PSUM.
#     fp16 hi/lo operand splits stacked 3x along the contract dim give
#     near-fp32 precision at full PE speed (matmul cost ~ free size only).
#   - ACT exp: PSUM[128,1024] -> SBUF fp16 (scalar engine is the ~55us floor).
#   - PV-matmul: lhsT=[128,7]=[pc_hi,pc_lo,1], rhs=wexp[128,512], accumulated
#     over all 32 p-tiles into one PSUM bank (4 col-group regions, one per
#     512-site tile).
# Host combines the 8 cores' partial sums (offset cancels in the ratio).

import os
import sys

for _p in ("/opt/trn_rl_repo", "/root/.axon_site/_ro/trn_rl_repo"):
    if os.path.isdir(_p) and _p not in sys.path:
        sys.path.insert(0, _p)

import numpy as np

N_CORES = 8

_PROGRAM_CACHE = {}


def _split16(v):
    hi = v.astype(np.float16)
    lo = (v - hi.astype(np.float32)).astype(np.float16)
    return hi, lo


def _preprocess(x, images, mu, sigma, t):
    x = np.ascontiguousarray(np.asarray(x, np.float32))
    images = np.asarray(images, np.float32)
    t = int(np.asarray(t))
    mu_t = float(np.asarray(mu)[t])
    sig_t = float(np.asarray(sigma)[t])
    s2 = sig_t * sig_t
    a = mu_t / s2
    bconst = mu_t * mu_t / (2.0 * s2)
    B, C, H, W = x.shape
    imgs = images.reshape(-1, C, H, W)
    N_all = imgs.shape[0]
    S = B * H * W
    K = 3

    # dataset patches [P, 27], flatten order (c, di, dj); zero padding
    pz = np.pad(imgs, ((0, 0), (0, 0), (1, 1), (1, 1)))
    wins = np.empty((N_all, C, K, K, H, W), np.float32)
    for di in range(K):
        for dj in range(K):
            wins[:, :, di, dj] = pz[:, :, di:di + H, dj:dj + W]
    patches = wins.transpose(0, 4, 5, 1, 2, 3).reshape(N_all * H * W, C * K * K)
    P = patches.shape[0]
    pcent = patches[:, [4, 13, 22]]  # (c, di=1, dj=1) -> c*9+4
    pnorms = (patches.astype(np.float64) ** 2).sum(1).astype(np.float32)

    # x-side windows [S, 27], s = (b, y, x); circular padding
    xp = np.pad(x, ((0, 0), (0, 0), (1, 1), (1, 1)), mode="wrap")
    xwins = np.empty((B, C, K, K, H, W), np.float32)
    for di in range(K):
        for dj in range(K):
            xwins[:, :, di, dj] = xp[:, :, di:di + H, dj:dj + W]
    Xw = xwins.transpose(0, 4, 5, 1, 2, 3).reshape(S, C * K * K)
    x_norms = Xw.sum(1) ** 2
    xn2 = np.sqrt((Xw.astype(np.float64) ** 2).sum(1)).astype(np.float32)

    # per-site upper bound on max_p w, shifted down so wexp peaks near e^9
    Mt = (abs(a) * xn2 * np.sqrt(pnorms.max())
          - x_norms / (2 * s2) - bconst * pnorms.min() - 9.0).astype(np.float32)

    Xmat = np.empty((29, S), np.float32)
    Xmat[0:27] = Xw.T * a
    Xmat[27] = 1.0
    Xmat[28] = -x_norms / (2 * s2) - Mt
    Pmat = np.empty((29, P), np.float32)
    Pmat[0:27] = patches.T
    Pmat[27] = -bconst * pnorms
    Pmat[28] = 1.0

    Xh, Xl = _split16(Xmat)
    Ph, Pl = _split16(Pmat)
    xmat_hl = np.ascontiguousarray(np.concatenate([Xh, Xh, Xl], 0))   # [87, S]
    pmat_hl = np.ascontiguousarray(np.concatenate([Ph, Pl, Ph], 0))   # [87, P]

    pch, pcl = _split16(pcent)
    pc_aug = np.concatenate(
        [pch, pcl, np.ones((P, 1), np.float16)], 1)                    # [P, 7]

    return dict(xmat_hl=xmat_hl, pmat_hl=pmat_hl, pc_aug=pc_aug,
                mu_t=mu_t, s2=s2, x=x, B=B, C=C, H=H, W=W, S=S, P=P)


def _build_program(S, P_core, repeat=1):
    import concourse.bacc as bacc
    import concourse.mybir as mybir
    import concourse.tile as tile

    f16 = mybir.dt.float16
    f32 = mybir.dt.float32
    NT = P_core // 128          # p-tiles per core (32)
    NS = S // 512               # site tiles (4)
    assert NS == 4 and NT % 8 == 0

    nc = bacc.Bacc("TRN2", target_bir_lowering=False, debug=False,
                   num_devices=N_CORES)
    xmat_d = nc.declare_dram_parameter("xmat", (87, S), f16, isOutput=False)
    pmats_d = nc.declare_dram_parameter("pmats", (87, P_core), f16,
                                        isOutput=False)
    pcents_d = nc.declare_dram_parameter("pcents", (128, NT * 7), f16,
                                         isOutput=False)
    rout_d = nc.declare_dram_parameter("r_out", (NS * 7, 512), f32,
                                       isOutput=True)

    with tile.TileContext(nc) as tc:
        with tc.tile_pool(name="const", bufs=1) as const, \
             tc.tile_pool(name="wexp", bufs=3) as wpool, \
             tc.tile_pool(name="psw", bufs=3, space="PSUM") as psw, \
             tc.tile_pool(name="psr", bufs=1, space="PSUM") as psr:

            # warm the exp table while DMAs stream
            dummy = const.tile([128, 1], f32, tag="dummy")
            nc.vector.memset(dummy[:], 0.0)
            nc.scalar.activation(dummy[:], dummy[:],
                                 mybir.ActivationFunctionType.Exp)

            xmat_t = const.tile([87, S], f16, tag="xmat")
            nc.sync.dma_start(out=xmat_t[:], in_=xmat_d[:])
            pc_t = const.tile([128, NT * 7], f16, tag="pc")
            nc.sync.dma_start(out=pc_t[:], in_=pcents_d[:])
            pm_t = []
            chunk = NT // 4 * 128
            for q in range(4):
                pt = const.tile([87, chunk], f16, tag=f"pm{q}")
                nc.sync.dma_start(out=pt[:], in_=pmats_d[:, q * chunk:(q + 1) * chunk])
                pm_t.append(pt)

            # base_partition is restricted to {0,32,64}: s-tiles 0-2 go in
            # one bank at those bases, s-tile 3 in a second bank at base 0
            R_a = psr.tile([71, 512], f32, tag="Ra")
            R_b = psr.tile([7, 512], f32, tag="Rb")

            def r_slice(j):
                return R_a[32 * j:32 * j + 7, :] if j < 3 else R_b[0:7, :]
            for rep in range(repeat):
                for i in range(NT):
                    lhs = pm_t[i // (NT // 4)]
                    ci = (i % (NT // 4)) * 128
                    lhs_ap = lhs[:, ci:ci + 128]
                    for h in range(2):
                        wps = psw.tile([128, 1024], f32, tag="w")
                        for jj in range(2):
                            j = 2 * h + jj
                            nc.tensor.matmul(
                                wps[:, 512 * jj:512 * (jj + 1)],
                                lhs_ap,
                                xmat_t[:, 512 * j:512 * (j + 1)],
                                start=True, stop=True)
                        we = wpool.tile([128, 1024], f16, tag="we")
                        nc.scalar.activation(we[:], wps[:],
                                             mybir.ActivationFunctionType.Exp)
                        for jj in range(2):
                            j = 2 * h + jj
                            nc.tensor.matmul(
                                r_slice(j),
                                pc_t[:, 7 * i:7 * i + 7],
                                we[:, 512 * jj:512 * (jj + 1)],
                                start=(i == 0), stop=(i == NT - 1),
                                skip_group_check=True)
                if rep == repeat - 1:
                    # engine partition bases must be 32-aligned; DMA is not
                    r_sb = const.tile([128, 512], f32, tag="r_sb")
                    for j in range(NS):
                        nc.vector.tensor_copy(r_sb[32 * j:32 * j + 7, :],
                                              r_slice(j))
                        nc.sync.dma_start(out=rout_d[7 * j:7 * j + 7, :],
                                          in_=r_sb[32 * j:32 * j + 7, :])
    nc.compile()
    return nc


def _get_program(S, P_core, repeat=1):
    key = (S, P_core, repeat)
    if key not in _PROGRAM_CACHE:
        _PROGRAM_CACHE[key] = _build_program(S, P_core, repeat)
    return _PROGRAM_CACHE[key]


def _make_in_maps(d):
    P_core = d["P"] // N_CORES
    NT = P_core // 128
    in_maps = []
    for c in range(N_CORES):
        sl = slice(c * P_core, (c + 1) * P_core)
        pc_block = d["pc_aug"][sl].reshape(NT, 128, 7)
        pc_core = np.ascontiguousarray(
            pc_block.transpose(1, 0, 2).reshape(128, NT * 7))
        in_maps.append({
            "xmat": d["xmat_hl"],
            "pmats": np.ascontiguousarray(d["pmat_hl"][:, sl]),
            "pcents": pc_core,
        })
    return in_maps


def _postprocess(d, results):
    S, C, B, H, W = d["S"], d["C"], d["B"], d["H"], d["W"]
    R = np.zeros((28, 512), np.float64)
    for c in range(N_CORES):
        R += results[c]["r_out"].astype(np.float64)
    R = R.reshape(4, 7, 512)
    Rc = (R[:, 0:3, :] + R[:, 3:6, :]).transpose(1, 0, 2).reshape(C, S)
    sw = R[:, 6, :].reshape(S)
    xs = d["x"].transpose(1, 0, 2, 3).reshape(C, S)
    out = (d["mu_t"] * Rc / sw - xs) / d["s2"]
    return np.ascontiguousarray(
        out.reshape(C, B, H, W).transpose(1, 0, 2, 3)).astype(np.float32)


def kernel(x, images, mu, sigma, t):
    from concourse.bass_utils import run_bass_kernel_spmd

    d = _preprocess(x, images, mu, sigma, t)
    assert d["P"] % (N_CORES * 128) == 0
    nc = _get_program(d["S"], d["P"] // N_CORES)
    res = run_bass_kernel_spmd(nc, _make_in_maps(d), list(range(N_CORES)))
    return _postprocess(d, res.results)


# revision 21
# speedup vs baseline: 13.3551x; 13.3551x over previous
# Trainium2 Bass kernel for nn_EquivariantLocalScoreMachine (retrieval_knn).
#
# Math: for each spatial site s=(b,y,x) (S=2048) and dataset patch p (P=32768):
#   w[p,s] = (mu*conv[p,s] - (x_norms[s] + mu^2*pnorms[p])/2) / sigma^2
#   out[c,s] = (mu * sum_p e^w*pcent[p,c] / sum_p e^w - x[c,s]) / sigma^2
# The output is invariant to any per-site offset of w, so instead of an exact
# online max we fold a cheap Cauchy-Schwarz upper bound M~[s] (computed on the
# host) into the matmul and do a single pass.
#
# Device kernel (per core, patches sharded 8 ways -> 4096 patches/core):
#   - w-matmul: lhsT=[87,128] patch tile, rhs=[87,512] x-side tile -> PSUM.
#     fp16 hi/lo operand splits stacked 3x along the contract dim give
#     near-fp32 precision at full PE speed (matmul cost ~ free size only).
#   - ACT exp: PSUM[128,1024] -> SBUF fp16 (scalar engine is the ~55us floor).
#   - PV-matmul: lhsT=[128,7]=[pc_hi,pc_lo,1], rhs=wexp[128,512], accumulated
#     over all 32 p-tiles into one PSUM bank (4 col-group regions, one per
#     512-site tile).
# Host combines the 8 cores' partial sums (offset cancels in the ratio).

import os
import sys

for _p in ("/opt/trn_rl_repo", "/root/.axon_site/_ro/trn_rl_repo"):
    if os.path.isdir(_p) and _p not in sys.path:
        sys.path.insert(0, _p)

import numpy as np

N_CORES = 8

_PROGRAM_CACHE = {}


def _split16(v):
    hi = v.astype(np.float16)
    lo = (v - hi.astype(np.float32)).astype(np.float16)
    return hi, lo


def _preprocess(x, images, mu, sigma, t):
    x = np.ascontiguousarray(np.asarray(x, np.float32))
    images = np.asarray(images, np.float32)
    t = int(np.asarray(t))
    mu_t = float(np.asarray(mu)[t])
    sig_t = float(np.asarray(sigma)[t])
    s2 = sig_t * sig_t
    a = mu_t / s2
    bconst = mu_t * mu_t / (2.0 * s2)
    B, C, H, W = x.shape
    imgs = images.reshape(-1, C, H, W)
    N_all = imgs.shape[0]
    S = B * H * W
    K = 3

    # dataset patches [P, 27], flatten order (c, di, dj); zero padding
    pz = np.pad(imgs, ((0, 0), (0, 0), (1, 1), (1, 1)))
    wins = np.empty((N_all, C, K, K, H, W), np.float32)
    for di in range(K):
        for dj in range(K):
            wins[:, :, di, dj] = pz[:, :, di:di + H, dj:dj + W]
    patches = wins.transpose(0, 4, 5, 1, 2, 3).reshape(N_all * H * W, C * K * K)
    P = patches.shape[0]
    pcent = patches[:, [4, 13, 22]]  # (c, di=1, dj=1) -> c*9+4
    pnorms = (patches.astype(np.float64) ** 2).sum(1).astype(np.float32)

    # x-side windows [S, 27], s = (b, y, x); circular padding
    xp = np.pad(x, ((0, 0), (0, 0), (1, 1), (1, 1)), mode="wrap")
    xwins = np.empty((B, C, K, K, H, W), np.float32)
    for di in range(K):
        for dj in range(K):
            xwins[:, :, di, dj] = xp[:, :, di:di + H, dj:dj + W]
    Xw = xwins.transpose(0, 4, 5, 1, 2, 3).reshape(S, C * K * K)
    x_norms = Xw.sum(1) ** 2
    xn2 = np.sqrt((Xw.astype(np.float64) ** 2).sum(1)).astype(np.float32)

    # per-site upper bound on max_p w, shifted down so wexp peaks near e^9
    Mt = (abs(a) * xn2 * np.sqrt(pnorms.max())
          - x_norms / (2 * s2) - bconst * pnorms.min() - 9.0).astype(np.float32)

    Xmat = np.empty((29, S), np.float32)
    Xmat[0:27] = Xw.T * a
    Xmat[27] = 1.0
    Xmat[28] = -x_norms / (2 * s2) - Mt
    Pmat = np.empty((29, P), np.float32)
    Pmat[0:27] = patches.T
    Pmat[27] = -bconst * pnorms
    Pmat[28] = 1.0

    Xh, Xl = _split16(Xmat)
    Ph, Pl = _split16(Pmat)
    xmat_hl = np.ascontiguousarray(np.concatenate([Xh, Xh, Xl], 0))   # [87, S]
    pmat_hl = np.ascontiguousarray(np.concatenate([Ph, Pl, Ph], 0))   # [87, P]

    pch, pcl = _split16(pcent)
    pc_aug = np.concatenate(
        [pch, pcl, np.ones((P, 1), np.float16)], 1)                    # [P, 7]

    return dict(xmat_hl=xmat_hl, pmat_hl=pmat_hl, pc_aug=pc_aug,
                mu_t=mu_t, s2=s2, x=x, B=B, C=C, H=H, W=W, S=S, P=P)


def _build_program(S, P_core, repeat=1, fuse_ldw=False, loop_n=None,
                   skew=0, we_bufs=3):
    import concourse.bacc as bacc
    import concourse.mybir as mybir
    import concourse.tile as tile

    f16 = mybir.dt.float16
    f32 = mybir.dt.float32
    NT = P_core // 128          # p-tiles per core (32)
    NS = S // 512               # site tiles (4)
    assert NS == 4 and NT % 8 == 0

    nc = bacc.Bacc("TRN2", target_bir_lowering=False, debug=False,
                   num_devices=N_CORES)
    xmat_d = nc.declare_dram_parameter("xmat", (87, S), f16, isOutput=False)
    pmats_d = nc.declare_dram_parameter("pmats", (87, P_core), f16,
                                        isOutput=False)
    pcents_d = nc.declare_dram_parameter("pcents", (128, NT * 7), f16,
                                         isOutput=False)
    rout_d = nc.declare_dram_parameter("r_out", (NS * 7, 512), f32,
                                       isOutput=True)

    with tile.TileContext(nc) as tc:
        with tc.tile_pool(name="const", bufs=1) as const, \
             tc.tile_pool(name="wexp", bufs=we_bufs) as wpool, \
             tc.tile_pool(name="psw", bufs=3, space="PSUM") as psw, \
             tc.tile_pool(name="psr", bufs=1, space="PSUM") as psr:

            # warm the exp table while DMAs stream
            dummy = const.tile([128, 1], f32, tag="dummy")
            nc.vector.memset(dummy[:], 0.0)
            nc.scalar.activation(dummy[:], dummy[:],
                                 mybir.ActivationFunctionType.Exp)

            xmat_t = const.tile([87, S], f16, tag="xmat")
            nc.sync.dma_start(out=xmat_t[:], in_=xmat_d[:])
            pc_t = const.tile([128, NT * 7], f16, tag="pc")
            nc.sync.dma_start(out=pc_t[:], in_=pcents_d[:])
            pm_t = []
            chunk = NT // 4 * 128
            for q in range(4):
                pt = const.tile([87, chunk], f16, tag=f"pm{q}")
                nc.sync.dma_start(out=pt[:], in_=pmats_d[:, q * chunk:(q + 1) * chunk])
                pm_t.append(pt)

            # base_partition is restricted to {0,32,64}: s-tiles 0-2 go in
            # one bank at those bases, s-tile 3 in a second bank at base 0
            R_a = psr.tile([71, 512], f32, tag="Ra")
            R_b = psr.tile([7, 512], f32, tag="Rb")

            def r_slice(j):
                return R_a[32 * j:32 * j + 7, :] if j < 3 else R_b[0:7, :]

            import contextlib
            loop_cm = (tc.For_i(0, loop_n, 1,
                               hint_engines=(mybir.EngineType.PE,
                                             mybir.EngineType.Activation))
                       if loop_n else contextlib.nullcontext())
            with loop_cm:
                _emit_body(nc, tc, mybir, NT, NS, repeat if not loop_n else 1,
                           fuse_ldw, pm_t, xmat_t, pc_t, wpool, psw, r_slice,
                           const, rout_d, f16, f32,
                           emit_out=not loop_n, skew=skew)
            if loop_n:
                r_sb = const.tile([128, 512], f32, tag="r_sb")
                for j in range(NS):
                    nc.vector.tensor_copy(r_sb[32 * j:32 * j + 7, :],
                                          r_slice(j))
                    nc.sync.dma_start(out=rout_d[7 * j:7 * j + 7, :],
                                      in_=r_sb[32 * j:32 * j + 7, :])
    nc.compile()
    return nc


def _emit_body(nc, tc, mybir, NT, NS, repeat, fuse_ldw, pm_t, xmat_t, pc_t,
               wpool, psw, r_slice, const, rout_d, f16, f32, emit_out=True,
               skew=0):
            def emit_pv(ent):
                i, h, we = ent
                for jj in range(2):
                    j = 2 * h + jj
                    nc.tensor.matmul(
                        r_slice(j),
                        pc_t[:, 7 * i:7 * i + 7],
                        we[:, 512 * jj:512 * (jj + 1)],
                        start=(i == 0), stop=(i == NT - 1),
                        skip_group_check=True)

            if skew:
                # software pipeline: PV matmuls trail the w matmuls by `skew`
                # half-tiles so the in-order PE never stalls waiting for ACT
                for rep in range(repeat):
                    pending = []
                    for i in range(NT):
                        lhs = pm_t[i // (NT // 4)]
                        ci = (i % (NT // 4)) * 128
                        lhs_ap = lhs[:, ci:ci + 128]
                        for h in range(2):
                            wps = psw.tile([128, 1024], f32, tag="w")
                            for jj in range(2):
                                j = 2 * h + jj
                                nc.tensor.matmul(
                                    wps[:, 512 * jj:512 * (jj + 1)],
                                    lhs_ap,
                                    xmat_t[:, 512 * j:512 * (j + 1)],
                                    start=True, stop=True)
                            we = wpool.tile([128, 1024], f16, tag="we")
                            nc.scalar.activation(
                                we[:], wps[:],
                                mybir.ActivationFunctionType.Exp)
                            pending.append((i, h, we))
                            if len(pending) > skew:
                                emit_pv(pending.pop(0))
                    for ent in pending:
                        emit_pv(ent)
                    if emit_out and rep == repeat - 1:
                        r_sb = const.tile([128, 512], f32, tag="r_sb")
                        for j in range(NS):
                            nc.vector.tensor_copy(r_sb[32 * j:32 * j + 7, :],
                                                  r_slice(j))
                            nc.sync.dma_start(
                                out=rout_d[7 * j:7 * j + 7, :],
                                in_=r_sb[32 * j:32 * j + 7, :])
                return
            for rep in range(repeat):
                for i in range(NT):
                    lhs = pm_t[i // (NT // 4)]
                    ci = (i % (NT // 4)) * 128
                    lhs_ap = lhs[:, ci:ci + 128]
                    if fuse_ldw:
                        # all 4 w-matmuls back-to-back (one pmat LDW), then
                        # the exps, then all 4 PV matmuls (one pc LDW)
                        wtiles = []
                        for h in range(2):
                            wps = psw.tile([128, 1024], f32, tag="w")
                            for jj in range(2):
                                j = 2 * h + jj
                                nc.tensor.matmul(
                                    wps[:, 512 * jj:512 * (jj + 1)],
                                    lhs_ap,
                                    xmat_t[:, 512 * j:512 * (j + 1)],
                                    start=True, stop=True)
                            wtiles.append(wps)
                        wes = []
                        for h in range(2):
                            we = wpool.tile([128, 1024], f16, tag="we")
                            nc.scalar.activation(
                                we[:], wtiles[h][:],
                                mybir.ActivationFunctionType.Exp)
                            wes.append(we)
                        for h in range(2):
                            for jj in range(2):
                                j = 2 * h + jj
                                nc.tensor.matmul(
                                    r_slice(j),
                                    pc_t[:, 7 * i:7 * i + 7],
                                    wes[h][:, 512 * jj:512 * (jj + 1)],
                                    start=(i == 0), stop=(i == NT - 1),
                                    skip_group_check=True)
                        continue
                    for h in range(2):
                        wps = psw.tile([128, 1024], f32, tag="w")
                        for jj in range(2):
                            j = 2 * h + jj
                            nc.tensor.matmul(
                                wps[:, 512 * jj:512 * (jj + 1)],
                                lhs_ap,
                                xmat_t[:, 512 * j:512 * (j + 1)],
                                start=True, stop=True)
                        we = wpool.tile([128, 1024], f16, tag="we")
                        nc.scalar.activation(we[:], wps[:],
                                             mybir.ActivationFunctionType.Exp)
                        for jj in range(2):
                            j = 2 * h + jj
                            nc.tensor.matmul(
                                r_slice(j),
                                pc_t[:, 7 * i:7 * i + 7],
                                we[:, 512 * jj:512 * (jj + 1)],
                                start=(i == 0), stop=(i == NT - 1),
                                skip_group_check=True)
                if emit_out and rep == repeat - 1:
                    # engine partition bases must be 32-aligned; DMA is not
                    r_sb = const.tile([128, 512], f32, tag="r_sb")
                    for j in range(NS):
                        nc.vector.tensor_copy(r_sb[32 * j:32 * j + 7, :],
                                              r_slice(j))
                        nc.sync.dma_start(out=rout_d[7 * j:7 * j + 7, :],
                                          in_=r_sb[32 * j:32 * j + 7, :])


def _build_program_v3(S, P_core, repeat=1, loop_n=None, skew_w=1, we_bufs=3):
    """7-slot PSUM ring: w-matmuls fill slots round-robin; exp runs over
    alternating 4-slot [128,2048] / 3-slot [128,1536] windows (amortizes the
    ~0.4us per-ACTIVATE overhead); PV windows trail by skew_w so the in-order
    PE never stalls on ACT. R lives in one PSUM bank at col-group bases
    {0,32,64,96} (96 needs explicit tile_position)."""
    import contextlib

    import concourse.bacc as bacc
    import concourse.mybir as mybir
    import concourse.tile as tile

    f16 = mybir.dt.float16
    f32 = mybir.dt.float32
    NT = P_core // 128
    NS = S // 512
    NG = NT * NS                # 128 s-tile matmul slots per iteration
    assert NS == 4 and NT % 8 == 0

    nc = bacc.Bacc("TRN2", target_bir_lowering=False, debug=False,
                   num_devices=N_CORES)
    xmat_d = nc.declare_dram_parameter("xmat", (87, S), f16, isOutput=False)
    pmats_d = nc.declare_dram_parameter("pmats", (87, P_core), f16,
                                        isOutput=False)
    pcents_d = nc.declare_dram_parameter("pcents", (128, NT * 7), f16,
                                         isOutput=False)
    rout_d = nc.declare_dram_parameter("r_out", (NS * 7, 512), f32,
                                       isOutput=True)

    with tile.TileContext(nc) as tc:
        with tc.tile_pool(name="const", bufs=1) as const, \
             tc.tile_pool(name="wexp", bufs=we_bufs) as wpool, \
             tc.tile_pool(name="psw", bufs=1, space="PSUM") as psw, \
             tc.tile_pool(name="psr", bufs=1, space="PSUM") as psr:

            dummy = const.tile([128, 1], f32, tag="dummy")
            nc.vector.memset(dummy[:], 0.0)
            nc.scalar.activation(dummy[:], dummy[:],
                                 mybir.ActivationFunctionType.Exp)

            xmat_t = const.tile([87, S], f16, tag="xmat")
            for q in range(4):
                nc.sync.dma_start(out=xmat_t[:, q * (S // 4):(q + 1) * (S // 4)],
                                  in_=xmat_d[:, q * (S // 4):(q + 1) * (S // 4)])
            pc_t = const.tile([128, NT * 7], f16, tag="pc")
            nc.sync.dma_start(out=pc_t[:], in_=pcents_d[:])
            pm_t = []
            chunk = NT // 4 * 128
            for q in range(4):
                pt = const.tile([87, chunk], f16, tag=f"pm{q}")
                nc.sync.dma_start(out=pt[:],
                                  in_=pmats_d[:, q * chunk:(q + 1) * chunk])
                pm_t.append(pt)

            # Each exp window must be its OWN psum tensor: Tile serializes
            # PE-writes vs ACT-reads within one PSUM tensor regardless of
            # bank. Alternate a 4-bank and a 3-bank tile (4+3+1(R)=8 banks).
            R = psr.tile([103, 512], f32, tag="R")              # 1 bank

            windows = []
            g = 0
            m = 0
            while g < NG:
                n = min(4 if m % 2 == 0 else 3, NG - g)
                windows.append((m % 2, n, g))
                g += n
                m += 1

            loop_cm = (tc.For_i(0, loop_n, 1,
                                hint_engines=(mybir.EngineType.PE,
                                              mybir.EngineType.Activation))
                       if loop_n else contextlib.nullcontext())
            with loop_cm:
                for rep in range(repeat if not loop_n else 1):
                    pending = []

                    def emit_pv(ent):
                        n, g0, we = ent
                        for k in range(n):
                            gg = g0 + k
                            i, j = gg // NS, gg % NS
                            nc.tensor.matmul(
                                R[32 * j:32 * j + 7, :],
                                pc_t[:, 7 * i:7 * i + 7],
                                we[:, 512 * k:512 * (k + 1)],
                                start=(i == 0), stop=(i == NT - 1),
                                skip_group_check=True,
                                tile_position=(0, 32 * j))

                    for (par, n, g0) in windows:
                        wt = psw.tile([128, 2048 if par == 0 else 1536],
                                      f32, tag=f"w{par}")
                        for k in range(n):
                            gg = g0 + k
                            i, j = gg // NS, gg % NS
                            lhs = pm_t[i // (NT // 4)]
                            ci = (i % (NT // 4)) * 128
                            nc.tensor.matmul(
                                wt[:, 512 * k:512 * (k + 1)],
                                lhs[:, ci:ci + 128],
                                xmat_t[:, 512 * j:512 * (j + 1)],
                                start=True, stop=True)
                        we = wpool.tile([128, 2048 if par == 0 else 1536],
                                        f16, tag=f"we{par}")
                        nc.scalar.activation(
                            we[:, 0:512 * n], wt[:, 0:512 * n],
                            mybir.ActivationFunctionType.Exp)
                        pending.append((n, g0, we))
                        if len(pending) > skew_w:
                            emit_pv(pending.pop(0))
                    for ent in pending:
                        emit_pv(ent)
            r_sb = const.tile([128, 512], f32, tag="r_sb")
            for j in range(NS):
                nc.vector.tensor_copy(r_sb[32 * j:32 * j + 7, :],
                                      R[32 * j:32 * j + 7, :])
                nc.sync.dma_start(out=rout_d[7 * j:7 * j + 7, :],
                                  in_=r_sb[32 * j:32 * j + 7, :])
    nc.compile()
    return nc


def _get_program(S, P_core, repeat=1, fuse_ldw=False, loop_n=None,
                 skew=0, we_bufs=3):
    key = (S, P_core, repeat, fuse_ldw, loop_n, skew, we_bufs)
    if key not in _PROGRAM_CACHE:
        _PROGRAM_CACHE[key] = _build_program(S, P_core, repeat, fuse_ldw,
                                             loop_n, skew, we_bufs)
    return _PROGRAM_CACHE[key]


def _get_program_best(S, P_core, loop_n=None):
    key = ("best", S, P_core, loop_n)
    if key not in _PROGRAM_CACHE:
        _PROGRAM_CACHE[key] = _build_program_v3(S, P_core, loop_n=loop_n,
                                                skew_w=2, we_bufs=2)
    return _PROGRAM_CACHE[key]


def _make_in_maps(d):
    P_core = d["P"] // N_CORES
    NT = P_core // 128
    in_maps = []
    for c in range(N_CORES):
        sl = slice(c * P_core, (c + 1) * P_core)
        pc_block = d["pc_aug"][sl].reshape(NT, 128, 7)
        pc_core = np.ascontiguousarray(
            pc_block.transpose(1, 0, 2).reshape(128, NT * 7))
        in_maps.append({
            "xmat": d["xmat_hl"],
            "pmats": np.ascontiguousarray(d["pmat_hl"][:, sl]),
            "pcents": pc_core,
        })
    return in_maps


def _postprocess(d, results):
    S, C, B, H, W = d["S"], d["C"], d["B"], d["H"], d["W"]
    R = np.zeros((28, 512), np.float64)
    for c in range(N_CORES):
        R += results[c]["r_out"].astype(np.float64)
    R = R.reshape(4, 7, 512)
    Rc = (R[:, 0:3, :] + R[:, 3:6, :]).transpose(1, 0, 2).reshape(C, S)
    sw = R[:, 6, :].reshape(S)
    xs = d["x"].transpose(1, 0, 2, 3).reshape(C, S)
    out = (d["mu_t"] * Rc / sw - xs) / d["s2"]
    return np.ascontiguousarray(
        out.reshape(C, B, H, W).transpose(1, 0, 2, 3)).astype(np.float32)


def kernel(x, images, mu, sigma, t):
    from concourse.bass_utils import run_bass_kernel_spmd

    d = _preprocess(x, images, mu, sigma, t)
    assert d["P"] % (N_CORES * 128) == 0
    nc = _get_program_best(d["S"], d["P"] // N_CORES)
    res = run_bass_kernel_spmd(nc, _make_in_maps(d), list(range(N_CORES)))
    return _postprocess(d, res.results)


# revision 22
# speedup vs baseline: 14.5837x; 1.0920x over previous
# Trainium2 Bass kernel for nn_EquivariantLocalScoreMachine (retrieval_knn).
#
# Math: for each spatial site s=(b,y,x) (S=2048) and dataset patch p (P=32768):
#   w[p,s] = (mu*conv[p,s] - (x_norms[s] + mu^2*pnorms[p])/2) / sigma^2
#   out[c,s] = (mu * sum_p e^w*pcent[p,c] / sum_p e^w - x[c,s]) / sigma^2
# The output is invariant to any per-site offset of w, so instead of an exact
# online max we fold a cheap Cauchy-Schwarz upper bound M~[s] (computed on the
# host) into the matmul and do a single pass.
#
# Device kernel (per core, patches sharded 8 ways -> 4096 patches/core):
#   - w-matmul: lhsT=[87,128] patch tile, rhs=[87,512] x-side tile -> PSUM.
#     fp16 hi/lo operand splits stacked 3x along the contract dim give
#     near-fp32 precision at full PE speed (matmul cost ~ free size only).
#   - ACT exp: PSUM[128,1024] -> SBUF fp16 (scalar engine is the ~55us floor).
#   - PV-matmul: lhsT=[128,7]=[pc_hi,pc_lo,1], rhs=wexp[128,512], accumulated
#     over all 32 p-tiles into one PSUM bank (4 col-group regions, one per
#     512-site tile).
# Host combines the 8 cores' partial sums (offset cancels in the ratio).

import os
import sys

for _p in ("/opt/trn_rl_repo", "/root/.axon_site/_ro/trn_rl_repo"):
    if os.path.isdir(_p) and _p not in sys.path:
        sys.path.insert(0, _p)

import numpy as np

N_CORES = 8

_PROGRAM_CACHE = {}


def _split16(v):
    hi = v.astype(np.float16)
    lo = (v - hi.astype(np.float32)).astype(np.float16)
    return hi, lo


def _preprocess(x, images, mu, sigma, t):
    x = np.ascontiguousarray(np.asarray(x, np.float32))
    images = np.asarray(images, np.float32)
    t = int(np.asarray(t))
    mu_t = float(np.asarray(mu)[t])
    sig_t = float(np.asarray(sigma)[t])
    s2 = sig_t * sig_t
    a = mu_t / s2
    bconst = mu_t * mu_t / (2.0 * s2)
    B, C, H, W = x.shape
    imgs = images.reshape(-1, C, H, W)
    N_all = imgs.shape[0]
    S = B * H * W
    K = 3

    # dataset patches [P, 27], flatten order (c, di, dj); zero padding
    pz = np.pad(imgs, ((0, 0), (0, 0), (1, 1), (1, 1)))
    wins = np.empty((N_all, C, K, K, H, W), np.float32)
    for di in range(K):
        for dj in range(K):
            wins[:, :, di, dj] = pz[:, :, di:di + H, dj:dj + W]
    patches = wins.transpose(0, 4, 5, 1, 2, 3).reshape(N_all * H * W, C * K * K)
    P = patches.shape[0]
    pcent = patches[:, [4, 13, 22]]  # (c, di=1, dj=1) -> c*9+4
    pnorms = (patches.astype(np.float64) ** 2).sum(1).astype(np.float32)

    # x-side windows [S, 27], s = (b, y, x); circular padding
    xp = np.pad(x, ((0, 0), (0, 0), (1, 1), (1, 1)), mode="wrap")
    xwins = np.empty((B, C, K, K, H, W), np.float32)
    for di in range(K):
        for dj in range(K):
            xwins[:, :, di, dj] = xp[:, :, di:di + H, dj:dj + W]
    Xw = xwins.transpose(0, 4, 5, 1, 2, 3).reshape(S, C * K * K)
    x_norms = Xw.sum(1) ** 2
    xn2 = np.sqrt((Xw.astype(np.float64) ** 2).sum(1)).astype(np.float32)

    # per-site upper bound on max_p w, shifted down so wexp peaks near e^9
    Mt = (abs(a) * xn2 * np.sqrt(pnorms.max())
          - x_norms / (2 * s2) - bconst * pnorms.min() - 9.0).astype(np.float32)

    Xmat = np.empty((29, S), np.float32)
    Xmat[0:27] = Xw.T * a
    Xmat[27] = 1.0
    Xmat[28] = -x_norms / (2 * s2) - Mt
    Pmat = np.empty((29, P), np.float32)
    Pmat[0:27] = patches.T
    Pmat[27] = -bconst * pnorms
    Pmat[28] = 1.0

    Xh, Xl = _split16(Xmat)
    Ph, Pl = _split16(Pmat)
    xmat_hl = np.ascontiguousarray(np.concatenate([Xh, Xh, Xl], 0))   # [87, S]
    pmat_hl = np.ascontiguousarray(np.concatenate([Ph, Pl, Ph], 0))   # [87, P]

    pch, pcl = _split16(pcent)
    pc_aug = np.concatenate(
        [pch, pcl, np.ones((P, 1), np.float16)], 1)                    # [P, 7]

    return dict(xmat_hl=xmat_hl, pmat_hl=pmat_hl, pc_aug=pc_aug,
                mu_t=mu_t, s2=s2, x=x, B=B, C=C, H=H, W=W, S=S, P=P)


def _build_program(S, P_core, repeat=1, fuse_ldw=False, loop_n=None,
                   skew=0, we_bufs=3):
    import concourse.bacc as bacc
    import concourse.mybir as mybir
    import concourse.tile as tile

    f16 = mybir.dt.float16
    f32 = mybir.dt.float32
    NT = P_core // 128          # p-tiles per core (32)
    NS = S // 512               # site tiles (4)
    assert NS == 4 and NT % 8 == 0

    nc = bacc.Bacc("TRN2", target_bir_lowering=False, debug=False,
                   num_devices=N_CORES)
    xmat_d = nc.declare_dram_parameter("xmat", (87, S), f16, isOutput=False)
    pmats_d = nc.declare_dram_parameter("pmats", (87, P_core), f16,
                                        isOutput=False)
    pcents_d = nc.declare_dram_parameter("pcents", (128, NT * 7), f16,
                                         isOutput=False)
    rout_d = nc.declare_dram_parameter("r_out", (NS * 7, 512), f32,
                                       isOutput=True)

    with tile.TileContext(nc) as tc:
        with tc.tile_pool(name="const", bufs=1) as const, \
             tc.tile_pool(name="wexp", bufs=we_bufs) as wpool, \
             tc.tile_pool(name="psw", bufs=3, space="PSUM") as psw, \
             tc.tile_pool(name="psr", bufs=1, space="PSUM") as psr:

            # warm the exp table while DMAs stream
            dummy = const.tile([128, 1], f32, tag="dummy")
            nc.vector.memset(dummy[:], 0.0)
            nc.scalar.activation(dummy[:], dummy[:],
                                 mybir.ActivationFunctionType.Exp)

            xmat_t = const.tile([87, S], f16, tag="xmat")
            nc.sync.dma_start(out=xmat_t[:], in_=xmat_d[:])
            pc_t = const.tile([128, NT * 7], f16, tag="pc")
            nc.sync.dma_start(out=pc_t[:], in_=pcents_d[:])
            pm_t = []
            chunk = NT // 4 * 128
            for q in range(4):
                pt = const.tile([87, chunk], f16, tag=f"pm{q}")
                nc.sync.dma_start(out=pt[:], in_=pmats_d[:, q * chunk:(q + 1) * chunk])
                pm_t.append(pt)

            # base_partition is restricted to {0,32,64}: s-tiles 0-2 go in
            # one bank at those bases, s-tile 3 in a second bank at base 0
            R_a = psr.tile([71, 512], f32, tag="Ra")
            R_b = psr.tile([7, 512], f32, tag="Rb")

            def r_slice(j):
                return R_a[32 * j:32 * j + 7, :] if j < 3 else R_b[0:7, :]

            import contextlib
            loop_cm = (tc.For_i(0, loop_n, 1,
                               hint_engines=(mybir.EngineType.PE,
                                             mybir.EngineType.Activation))
                       if loop_n else contextlib.nullcontext())
            with loop_cm:
                _emit_body(nc, tc, mybir, NT, NS, repeat if not loop_n else 1,
                           fuse_ldw, pm_t, xmat_t, pc_t, wpool, psw, r_slice,
                           const, rout_d, f16, f32,
                           emit_out=not loop_n, skew=skew)
            if loop_n:
                r_sb = const.tile([128, 512], f32, tag="r_sb")
                for j in range(NS):
                    nc.vector.tensor_copy(r_sb[32 * j:32 * j + 7, :],
                                          r_slice(j))
                    nc.sync.dma_start(out=rout_d[7 * j:7 * j + 7, :],
                                      in_=r_sb[32 * j:32 * j + 7, :])
    nc.compile()
    return nc


def _emit_body(nc, tc, mybir, NT, NS, repeat, fuse_ldw, pm_t, xmat_t, pc_t,
               wpool, psw, r_slice, const, rout_d, f16, f32, emit_out=True,
               skew=0):
            def emit_pv(ent):
                i, h, we = ent
                for jj in range(2):
                    j = 2 * h + jj
                    nc.tensor.matmul(
                        r_slice(j),
                        pc_t[:, 7 * i:7 * i + 7],
                        we[:, 512 * jj:512 * (jj + 1)],
                        start=(i == 0), stop=(i == NT - 1),
                        skip_group_check=True)

            if skew:
                # software pipeline: PV matmuls trail the w matmuls by `skew`
                # half-tiles so the in-order PE never stalls waiting for ACT
                for rep in range(repeat):
                    pending = []
                    for i in range(NT):
                        lhs = pm_t[i // (NT // 4)]
                        ci = (i % (NT // 4)) * 128
                        lhs_ap = lhs[:, ci:ci + 128]
                        for h in range(2):
                            wps = psw.tile([128, 1024], f32, tag="w")
                            for jj in range(2):
                                j = 2 * h + jj
                                nc.tensor.matmul(
                                    wps[:, 512 * jj:512 * (jj + 1)],
                                    lhs_ap,
                                    xmat_t[:, 512 * j:512 * (j + 1)],
                                    start=True, stop=True)
                            we = wpool.tile([128, 1024], f16, tag="we")
                            nc.scalar.activation(
                                we[:], wps[:],
                                mybir.ActivationFunctionType.Exp)
                            pending.append((i, h, we))
                            if len(pending) > skew:
                                emit_pv(pending.pop(0))
                    for ent in pending:
                        emit_pv(ent)
                    if emit_out and rep == repeat - 1:
                        r_sb = const.tile([128, 512], f32, tag="r_sb")
                        for j in range(NS):
                            nc.vector.tensor_copy(r_sb[32 * j:32 * j + 7, :],
                                                  r_slice(j))
                            nc.sync.dma_start(
                                out=rout_d[7 * j:7 * j + 7, :],
                                in_=r_sb[32 * j:32 * j + 7, :])
                return
            for rep in range(repeat):
                for i in range(NT):
                    lhs = pm_t[i // (NT // 4)]
                    ci = (i % (NT // 4)) * 128
                    lhs_ap = lhs[:, ci:ci + 128]
                    if fuse_ldw:
                        # all 4 w-matmuls back-to-back (one pmat LDW), then
                        # the exps, then all 4 PV matmuls (one pc LDW)
                        wtiles = []
                        for h in range(2):
                            wps = psw.tile([128, 1024], f32, tag="w")
                            for jj in range(2):
                                j = 2 * h + jj
                                nc.tensor.matmul(
                                    wps[:, 512 * jj:512 * (jj + 1)],
                                    lhs_ap,
                                    xmat_t[:, 512 * j:512 * (j + 1)],
                                    start=True, stop=True)
                            wtiles.append(wps)
                        wes = []
                        for h in range(2):
                            we = wpool.tile([128, 1024], f16, tag="we")
                            nc.scalar.activation(
                                we[:], wtiles[h][:],
                                mybir.ActivationFunctionType.Exp)
                            wes.append(we)
                        for h in range(2):
                            for jj in range(2):
                                j = 2 * h + jj
                                nc.tensor.matmul(
                                    r_slice(j),
                                    pc_t[:, 7 * i:7 * i + 7],
                                    wes[h][:, 512 * jj:512 * (jj + 1)],
                                    start=(i == 0), stop=(i == NT - 1),
                                    skip_group_check=True)
                        continue
                    for h in range(2):
                        wps = psw.tile([128, 1024], f32, tag="w")
                        for jj in range(2):
                            j = 2 * h + jj
                            nc.tensor.matmul(
                                wps[:, 512 * jj:512 * (jj + 1)],
                                lhs_ap,
                                xmat_t[:, 512 * j:512 * (j + 1)],
                                start=True, stop=True)
                        we = wpool.tile([128, 1024], f16, tag="we")
                        nc.scalar.activation(we[:], wps[:],
                                             mybir.ActivationFunctionType.Exp)
                        for jj in range(2):
                            j = 2 * h + jj
                            nc.tensor.matmul(
                                r_slice(j),
                                pc_t[:, 7 * i:7 * i + 7],
                                we[:, 512 * jj:512 * (jj + 1)],
                                start=(i == 0), stop=(i == NT - 1),
                                skip_group_check=True)
                if emit_out and rep == repeat - 1:
                    # engine partition bases must be 32-aligned; DMA is not
                    r_sb = const.tile([128, 512], f32, tag="r_sb")
                    for j in range(NS):
                        nc.vector.tensor_copy(r_sb[32 * j:32 * j + 7, :],
                                              r_slice(j))
                        nc.sync.dma_start(out=rout_d[7 * j:7 * j + 7, :],
                                          in_=r_sb[32 * j:32 * j + 7, :])


def _build_program_v3(S, P_core, repeat=1, loop_n=None, skew_w=1, we_bufs=3,
                      stag=False):
    """7-slot PSUM ring: w-matmuls fill slots round-robin; exp runs over
    alternating 4-slot [128,2048] / 3-slot [128,1536] windows (amortizes the
    ~0.4us per-ACTIVATE overhead); PV windows trail by skew_w so the in-order
    PE never stalls on ACT. R lives in one PSUM bank at col-group bases
    {0,32,64,96} (96 needs explicit tile_position)."""
    import contextlib

    import concourse.bacc as bacc
    import concourse.mybir as mybir
    import concourse.tile as tile

    f16 = mybir.dt.float16
    f32 = mybir.dt.float32
    NT = P_core // 128
    NS = S // 512
    NG = NT * NS                # 128 s-tile matmul slots per iteration
    assert NS == 4 and NT % 8 == 0

    nc = bacc.Bacc("TRN2", target_bir_lowering=False, debug=False,
                   num_devices=N_CORES)
    xmat_d = nc.declare_dram_parameter("xmat", (87, S), f16, isOutput=False)
    pmats_d = nc.declare_dram_parameter("pmats", (87, P_core), f16,
                                        isOutput=False)
    pcents_d = nc.declare_dram_parameter("pcents", (128, NT * 7), f16,
                                         isOutput=False)
    rout_d = nc.declare_dram_parameter("r_out", (NS * 7, 512), f32,
                                       isOutput=True)

    with tile.TileContext(nc) as tc:
        with tc.tile_pool(name="const", bufs=1) as const, \
             tc.tile_pool(name="wexp", bufs=we_bufs) as wpool, \
             tc.tile_pool(name="psw", bufs=1, space="PSUM") as psw, \
             tc.tile_pool(name="psr", bufs=1, space="PSUM") as psr:

            dummy = const.tile([128, 1], f32, tag="dummy")
            nc.vector.memset(dummy[:], 0.0)
            nc.scalar.activation(dummy[:], dummy[:],
                                 mybir.ActivationFunctionType.Exp)

            xmat_t = const.tile([87, S], f16, tag="xmat")
            for q in range(4):
                nc.sync.dma_start(out=xmat_t[:, q * (S // 4):(q + 1) * (S // 4)],
                                  in_=xmat_d[:, q * (S // 4):(q + 1) * (S // 4)])
            pc_t = const.tile([128, NT * 7], f16, tag="pc")
            nc.sync.dma_start(out=pc_t[:], in_=pcents_d[:])
            pm_t = []
            chunk = NT // 4 * 128
            for q in range(4):
                pt = const.tile([87, chunk], f16, tag=f"pm{q}")
                nc.sync.dma_start(out=pt[:],
                                  in_=pmats_d[:, q * chunk:(q + 1) * chunk])
                pm_t.append(pt)

            # Each exp window must be its OWN psum tensor: Tile serializes
            # PE-writes vs ACT-reads within one PSUM tensor regardless of
            # bank. Alternate a 4-bank and a 3-bank tile (4+3+1(R)=8 banks).
            R = psr.tile([103, 512], f32, tag="R")              # 1 bank

            windows = []
            g = 0
            m = 0
            while g < NG:
                n = min(4 if m % 2 == 0 else 3, NG - g)
                windows.append((m % 2, n, g))
                g += n
                m += 1

            loop_cm = (tc.For_i(0, loop_n, 1,
                                hint_engines=(mybir.EngineType.PE,
                                              mybir.EngineType.Activation),
                                staggered_reset=stag)
                       if loop_n else contextlib.nullcontext())
            with loop_cm:
                for rep in range(repeat if not loop_n else 1):
                    pending = []

                    def emit_pv(ent):
                        n, g0, we = ent
                        for k in range(n):
                            gg = g0 + k
                            i, j = gg // NS, gg % NS
                            nc.tensor.matmul(
                                R[32 * j:32 * j + 7, :],
                                pc_t[:, 7 * i:7 * i + 7],
                                we[:, 512 * k:512 * (k + 1)],
                                start=(i == 0), stop=(i == NT - 1),
                                skip_group_check=True,
                                tile_position=(0, 32 * j))

                    for (par, n, g0) in windows:
                        wt = psw.tile([128, 2048 if par == 0 else 1536],
                                      f32, tag=f"w{par}")
                        for k in range(n):
                            gg = g0 + k
                            i, j = gg // NS, gg % NS
                            lhs = pm_t[i // (NT // 4)]
                            ci = (i % (NT // 4)) * 128
                            nc.tensor.matmul(
                                wt[:, 512 * k:512 * (k + 1)],
                                lhs[:, ci:ci + 128],
                                xmat_t[:, 512 * j:512 * (j + 1)],
                                start=True, stop=True)
                        we = wpool.tile([128, 2048 if par == 0 else 1536],
                                        f16, tag=f"we{par}")
                        nc.scalar.activation(
                            we[:, 0:512 * n], wt[:, 0:512 * n],
                            mybir.ActivationFunctionType.Exp)
                        pending.append((n, g0, we))
                        if len(pending) > skew_w:
                            emit_pv(pending.pop(0))
                    for ent in pending:
                        emit_pv(ent)
            r_sb = const.tile([128, 512], f32, tag="r_sb")
            for j in range(NS):
                nc.vector.tensor_copy(r_sb[32 * j:32 * j + 7, :],
                                      R[32 * j:32 * j + 7, :])
                nc.sync.dma_start(out=rout_d[7 * j:7 * j + 7, :],
                                  in_=r_sb[32 * j:32 * j + 7, :])
    nc.compile()
    return nc


def _get_program(S, P_core, repeat=1, fuse_ldw=False, loop_n=None,
                 skew=0, we_bufs=3):
    key = (S, P_core, repeat, fuse_ldw, loop_n, skew, we_bufs)
    if key not in _PROGRAM_CACHE:
        _PROGRAM_CACHE[key] = _build_program(S, P_core, repeat, fuse_ldw,
                                             loop_n, skew, we_bufs)
    return _PROGRAM_CACHE[key]


def _get_program_best(S, P_core, loop_n=None):
    key = ("best", S, P_core, loop_n)
    if key not in _PROGRAM_CACHE:
        _PROGRAM_CACHE[key] = _build_program_v3(S, P_core, loop_n=loop_n,
                                                skew_w=2, we_bufs=2)
    return _PROGRAM_CACHE[key]


def _make_in_maps(d):
    P_core = d["P"] // N_CORES
    NT = P_core // 128
    in_maps = []
    for c in range(N_CORES):
        sl = slice(c * P_core, (c + 1) * P_core)
        pc_block = d["pc_aug"][sl].reshape(NT, 128, 7)
        pc_core = np.ascontiguousarray(
            pc_block.transpose(1, 0, 2).reshape(128, NT * 7))
        in_maps.append({
            "xmat": d["xmat_hl"],
            "pmats": np.ascontiguousarray(d["pmat_hl"][:, sl]),
            "pcents": pc_core,
        })
    return in_maps


def _postprocess(d, results):
    S, C, B, H, W = d["S"], d["C"], d["B"], d["H"], d["W"]
    R = np.zeros((28, 512), np.float64)
    for c in range(N_CORES):
        R += results[c]["r_out"].astype(np.float64)
    R = R.reshape(4, 7, 512)
    Rc = (R[:, 0:3, :] + R[:, 3:6, :]).transpose(1, 0, 2).reshape(C, S)
    sw = R[:, 6, :].reshape(S)
    xs = d["x"].transpose(1, 0, 2, 3).reshape(C, S)
    out = (d["mu_t"] * Rc / sw - xs) / d["s2"]
    return np.ascontiguousarray(
        out.reshape(C, B, H, W).transpose(1, 0, 2, 3)).astype(np.float32)


def kernel(x, images, mu, sigma, t):
    from concourse.bass_utils import run_bass_kernel_spmd

    d = _preprocess(x, images, mu, sigma, t)
    assert d["P"] % (N_CORES * 128) == 0
    nc = _get_program_best(d["S"], d["P"] // N_CORES)
    res = run_bass_kernel_spmd(nc, _make_in_maps(d), list(range(N_CORES)))
    return _postprocess(d, res.results)


# revision 23
# speedup vs baseline: 17.2163x; 1.1805x over previous
# Trainium2 Bass kernel for nn_EquivariantLocalScoreMachine (retrieval_knn).
#
# Math: for each spatial site s=(b,y,x) (S=2048) and dataset patch p (P=32768):
#   w[p,s] = (mu*conv[p,s] - (x_norms[s] + mu^2*pnorms[p])/2) / sigma^2
#   out[c,s] = (mu * sum_p e^w*pcent[p,c] / sum_p e^w - x[c,s]) / sigma^2
# The output is invariant to any per-site offset of w, so instead of an exact
# online max we fold a cheap Cauchy-Schwarz upper bound M~[s] (computed on the
# host) into the matmul and do a single pass.
#
# Device kernel (per core, patches sharded 8 ways -> 4096 patches/core):
#   - w-matmul: lhsT=[87,128] patch tile, rhs=[87,512] x-side tile -> PSUM.
#     fp16 hi/lo operand splits stacked 3x along the contract dim give
#     near-fp32 precision at full PE speed (matmul cost ~ free size only).
#   - ACT exp: PSUM[128,1024] -> SBUF fp16 (scalar engine is the ~55us floor).
#   - PV-matmul: lhsT=[128,7]=[pc_hi,pc_lo,1], rhs=wexp[128,512], accumulated
#     over all 32 p-tiles into one PSUM bank (4 col-group regions, one per
#     512-site tile).
# Host combines the 8 cores' partial sums (offset cancels in the ratio).

import os
import sys

for _p in ("/opt/trn_rl_repo", "/root/.axon_site/_ro/trn_rl_repo"):
    if os.path.isdir(_p) and _p not in sys.path:
        sys.path.insert(0, _p)

import numpy as np

N_CORES = 8

_PROGRAM_CACHE = {}


def _split16(v):
    hi = v.astype(np.float16)
    lo = (v - hi.astype(np.float32)).astype(np.float16)
    return hi, lo


def _preprocess(x, images, mu, sigma, t):
    x = np.ascontiguousarray(np.asarray(x, np.float32))
    images = np.asarray(images, np.float32)
    t = int(np.asarray(t))
    mu_t = float(np.asarray(mu)[t])
    sig_t = float(np.asarray(sigma)[t])
    s2 = sig_t * sig_t
    a = mu_t / s2
    bconst = mu_t * mu_t / (2.0 * s2)
    B, C, H, W = x.shape
    imgs = images.reshape(-1, C, H, W)
    N_all = imgs.shape[0]
    S = B * H * W
    K = 3

    # dataset patches [P, 27], flatten order (c, di, dj); zero padding
    pz = np.pad(imgs, ((0, 0), (0, 0), (1, 1), (1, 1)))
    wins = np.empty((N_all, C, K, K, H, W), np.float32)
    for di in range(K):
        for dj in range(K):
            wins[:, :, di, dj] = pz[:, :, di:di + H, dj:dj + W]
    patches = wins.transpose(0, 4, 5, 1, 2, 3).reshape(N_all * H * W, C * K * K)
    P = patches.shape[0]
    pcent = patches[:, [4, 13, 22]]  # (c, di=1, dj=1) -> c*9+4
    pnorms = (patches.astype(np.float64) ** 2).sum(1).astype(np.float32)

    # x-side windows [S, 27], s = (b, y, x); circular padding
    xp = np.pad(x, ((0, 0), (0, 0), (1, 1), (1, 1)), mode="wrap")
    xwins = np.empty((B, C, K, K, H, W), np.float32)
    for di in range(K):
        for dj in range(K):
            xwins[:, :, di, dj] = xp[:, :, di:di + H, dj:dj + W]
    Xw = xwins.transpose(0, 4, 5, 1, 2, 3).reshape(S, C * K * K)
    x_norms = Xw.sum(1) ** 2
    xn2 = np.sqrt((Xw.astype(np.float64) ** 2).sum(1)).astype(np.float32)

    # per-site upper bound on max_p w, shifted down so wexp peaks near e^9
    Mt = (abs(a) * xn2 * np.sqrt(pnorms.max())
          - x_norms / (2 * s2) - bconst * pnorms.min() - 9.0).astype(np.float32)

    Xmat = np.empty((29, S), np.float32)
    Xmat[0:27] = Xw.T * a
    Xmat[27] = 1.0
    Xmat[28] = -x_norms / (2 * s2) - Mt
    Pmat = np.empty((29, P), np.float32)
    Pmat[0:27] = patches.T
    Pmat[27] = -bconst * pnorms
    Pmat[28] = 1.0

    Xh, Xl = _split16(Xmat)
    Ph, Pl = _split16(Pmat)
    xmat_hl = np.ascontiguousarray(np.concatenate([Xh, Xh, Xl], 0))   # [87, S]
    pmat_hl = np.ascontiguousarray(np.concatenate([Ph, Pl, Ph], 0))   # [87, P]

    pch, pcl = _split16(pcent)
    pc_aug = np.concatenate(
        [pch, pcl, np.ones((P, 1), np.float16)], 1)                    # [P, 7]

    return dict(xmat_hl=xmat_hl, pmat_hl=pmat_hl, pc_aug=pc_aug,
                mu_t=mu_t, s2=s2, x=x, B=B, C=C, H=H, W=W, S=S, P=P)


def _build_program(S, P_core, repeat=1, fuse_ldw=False, loop_n=None,
                   skew=0, we_bufs=3):
    import concourse.bacc as bacc
    import concourse.mybir as mybir
    import concourse.tile as tile

    f16 = mybir.dt.float16
    f32 = mybir.dt.float32
    NT = P_core // 128          # p-tiles per core (32)
    NS = S // 512               # site tiles (4)
    assert NS == 4 and NT % 8 == 0

    nc = bacc.Bacc("TRN2", target_bir_lowering=False, debug=False,
                   num_devices=N_CORES)
    xmat_d = nc.declare_dram_parameter("xmat", (87, S), f16, isOutput=False)
    pmats_d = nc.declare_dram_parameter("pmats", (87, P_core), f16,
                                        isOutput=False)
    pcents_d = nc.declare_dram_parameter("pcents", (128, NT * 7), f16,
                                         isOutput=False)
    rout_d = nc.declare_dram_parameter("r_out", (NS * 7, 512), f32,
                                       isOutput=True)

    with tile.TileContext(nc) as tc:
        with tc.tile_pool(name="const", bufs=1) as const, \
             tc.tile_pool(name="wexp", bufs=we_bufs) as wpool, \
             tc.tile_pool(name="psw", bufs=3, space="PSUM") as psw, \
             tc.tile_pool(name="psr", bufs=1, space="PSUM") as psr:

            # warm the exp table while DMAs stream
            dummy = const.tile([128, 1], f32, tag="dummy")
            nc.vector.memset(dummy[:], 0.0)
            nc.scalar.activation(dummy[:], dummy[:],
                                 mybir.ActivationFunctionType.Exp)

            xmat_t = const.tile([87, S], f16, tag="xmat")
            nc.sync.dma_start(out=xmat_t[:], in_=xmat_d[:])
            pc_t = const.tile([128, NT * 7], f16, tag="pc")
            nc.sync.dma_start(out=pc_t[:], in_=pcents_d[:])
            pm_t = []
            chunk = NT // 4 * 128
            for q in range(4):
                pt = const.tile([87, chunk], f16, tag=f"pm{q}")
                nc.sync.dma_start(out=pt[:], in_=pmats_d[:, q * chunk:(q + 1) * chunk])
                pm_t.append(pt)

            # base_partition is restricted to {0,32,64}: s-tiles 0-2 go in
            # one bank at those bases, s-tile 3 in a second bank at base 0
            R_a = psr.tile([71, 512], f32, tag="Ra")
            R_b = psr.tile([7, 512], f32, tag="Rb")

            def r_slice(j):
                return R_a[32 * j:32 * j + 7, :] if j < 3 else R_b[0:7, :]

            import contextlib
            loop_cm = (tc.For_i(0, loop_n, 1,
                               hint_engines=(mybir.EngineType.PE,
                                             mybir.EngineType.Activation))
                       if loop_n else contextlib.nullcontext())
            with loop_cm:
                _emit_body(nc, tc, mybir, NT, NS, repeat if not loop_n else 1,
                           fuse_ldw, pm_t, xmat_t, pc_t, wpool, psw, r_slice,
                           const, rout_d, f16, f32,
                           emit_out=not loop_n, skew=skew)
            if loop_n:
                r_sb = const.tile([128, 512], f32, tag="r_sb")
                for j in range(NS):
                    nc.vector.tensor_copy(r_sb[32 * j:32 * j + 7, :],
                                          r_slice(j))
                    nc.sync.dma_start(out=rout_d[7 * j:7 * j + 7, :],
                                      in_=r_sb[32 * j:32 * j + 7, :])
    nc.compile()
    return nc


def _emit_body(nc, tc, mybir, NT, NS, repeat, fuse_ldw, pm_t, xmat_t, pc_t,
               wpool, psw, r_slice, const, rout_d, f16, f32, emit_out=True,
               skew=0):
            def emit_pv(ent):
                i, h, we = ent
                for jj in range(2):
                    j = 2 * h + jj
                    nc.tensor.matmul(
                        r_slice(j),
                        pc_t[:, 7 * i:7 * i + 7],
                        we[:, 512 * jj:512 * (jj + 1)],
                        start=(i == 0), stop=(i == NT - 1),
                        skip_group_check=True)

            if skew:
                # software pipeline: PV matmuls trail the w matmuls by `skew`
                # half-tiles so the in-order PE never stalls waiting for ACT
                for rep in range(repeat):
                    pending = []
                    for i in range(NT):
                        lhs = pm_t[i // (NT // 4)]
                        ci = (i % (NT // 4)) * 128
                        lhs_ap = lhs[:, ci:ci + 128]
                        for h in range(2):
                            wps = psw.tile([128, 1024], f32, tag="w")
                            for jj in range(2):
                                j = 2 * h + jj
                                nc.tensor.matmul(
                                    wps[:, 512 * jj:512 * (jj + 1)],
                                    lhs_ap,
                                    xmat_t[:, 512 * j:512 * (j + 1)],
                                    start=True, stop=True)
                            we = wpool.tile([128, 1024], f16, tag="we")
                            nc.scalar.activation(
                                we[:], wps[:],
                                mybir.ActivationFunctionType.Exp)
                            pending.append((i, h, we))
                            if len(pending) > skew:
                                emit_pv(pending.pop(0))
                    for ent in pending:
                        emit_pv(ent)
                    if emit_out and rep == repeat - 1:
                        r_sb = const.tile([128, 512], f32, tag="r_sb")
                        for j in range(NS):
                            nc.vector.tensor_copy(r_sb[32 * j:32 * j + 7, :],
                                                  r_slice(j))
                            nc.sync.dma_start(
                                out=rout_d[7 * j:7 * j + 7, :],
                                in_=r_sb[32 * j:32 * j + 7, :])
                return
            for rep in range(repeat):
                for i in range(NT):
                    lhs = pm_t[i // (NT // 4)]
                    ci = (i % (NT // 4)) * 128
                    lhs_ap = lhs[:, ci:ci + 128]
                    if fuse_ldw:
                        # all 4 w-matmuls back-to-back (one pmat LDW), then
                        # the exps, then all 4 PV matmuls (one pc LDW)
                        wtiles = []
                        for h in range(2):
                            wps = psw.tile([128, 1024], f32, tag="w")
                            for jj in range(2):
                                j = 2 * h + jj
                                nc.tensor.matmul(
                                    wps[:, 512 * jj:512 * (jj + 1)],
                                    lhs_ap,
                                    xmat_t[:, 512 * j:512 * (j + 1)],
                                    start=True, stop=True)
                            wtiles.append(wps)
                        wes = []
                        for h in range(2):
                            we = wpool.tile([128, 1024], f16, tag="we")
                            nc.scalar.activation(
                                we[:], wtiles[h][:],
                                mybir.ActivationFunctionType.Exp)
                            wes.append(we)
                        for h in range(2):
                            for jj in range(2):
                                j = 2 * h + jj
                                nc.tensor.matmul(
                                    r_slice(j),
                                    pc_t[:, 7 * i:7 * i + 7],
                                    wes[h][:, 512 * jj:512 * (jj + 1)],
                                    start=(i == 0), stop=(i == NT - 1),
                                    skip_group_check=True)
                        continue
                    for h in range(2):
                        wps = psw.tile([128, 1024], f32, tag="w")
                        for jj in range(2):
                            j = 2 * h + jj
                            nc.tensor.matmul(
                                wps[:, 512 * jj:512 * (jj + 1)],
                                lhs_ap,
                                xmat_t[:, 512 * j:512 * (j + 1)],
                                start=True, stop=True)
                        we = wpool.tile([128, 1024], f16, tag="we")
                        nc.scalar.activation(we[:], wps[:],
                                             mybir.ActivationFunctionType.Exp)
                        for jj in range(2):
                            j = 2 * h + jj
                            nc.tensor.matmul(
                                r_slice(j),
                                pc_t[:, 7 * i:7 * i + 7],
                                we[:, 512 * jj:512 * (jj + 1)],
                                start=(i == 0), stop=(i == NT - 1),
                                skip_group_check=True)
                if emit_out and rep == repeat - 1:
                    # engine partition bases must be 32-aligned; DMA is not
                    r_sb = const.tile([128, 512], f32, tag="r_sb")
                    for j in range(NS):
                        nc.vector.tensor_copy(r_sb[32 * j:32 * j + 7, :],
                                              r_slice(j))
                        nc.sync.dma_start(out=rout_d[7 * j:7 * j + 7, :],
                                          in_=r_sb[32 * j:32 * j + 7, :])


def _build_program_v3(S, P_core, repeat=1, loop_n=None, skew_w=1, we_bufs=3,
                      stag=False):
    """7-slot PSUM ring: w-matmuls fill slots round-robin; exp runs over
    alternating 4-slot [128,2048] / 3-slot [128,1536] windows (amortizes the
    ~0.4us per-ACTIVATE overhead); PV windows trail by skew_w so the in-order
    PE never stalls on ACT. R lives in one PSUM bank at col-group bases
    {0,32,64,96} (96 needs explicit tile_position)."""
    import contextlib

    import concourse.bacc as bacc
    import concourse.mybir as mybir
    import concourse.tile as tile

    f16 = mybir.dt.float16
    f32 = mybir.dt.float32
    NT = P_core // 128
    NS = S // 512
    NG = NT * NS                # 128 s-tile matmul slots per iteration
    assert NS == 4 and NT % 8 == 0

    nc = bacc.Bacc("TRN2", target_bir_lowering=False, debug=False,
                   num_devices=N_CORES)
    xmat_d = nc.declare_dram_parameter("xmat", (87, S), f16, isOutput=False)
    pmats_d = nc.declare_dram_parameter("pmats", (87, P_core), f16,
                                        isOutput=False)
    pcents_d = nc.declare_dram_parameter("pcents", (128, NT * 7), f16,
                                         isOutput=False)
    rout_d = nc.declare_dram_parameter("r_out", (NS * 7, 512), f32,
                                       isOutput=True)

    with tile.TileContext(nc) as tc:
        with tc.tile_pool(name="const", bufs=1) as const, \
             tc.tile_pool(name="wexp", bufs=we_bufs) as wpool, \
             tc.tile_pool(name="psw", bufs=1, space="PSUM") as psw, \
             tc.tile_pool(name="psr", bufs=1, space="PSUM") as psr:

            dummy = const.tile([128, 1], f32, tag="dummy")
            nc.vector.memset(dummy[:], 0.0)
            nc.scalar.activation(dummy[:], dummy[:],
                                 mybir.ActivationFunctionType.Exp)

            xmat_t = const.tile([87, S], f16, tag="xmat")
            for q in range(4):
                nc.sync.dma_start(out=xmat_t[:, q * (S // 4):(q + 1) * (S // 4)],
                                  in_=xmat_d[:, q * (S // 4):(q + 1) * (S // 4)])
            pc_t = const.tile([128, NT * 7], f16, tag="pc")
            nc.sync.dma_start(out=pc_t[:], in_=pcents_d[:])
            pm_t = []
            chunk = NT // 4 * 128
            for q in range(4):
                pt = const.tile([87, chunk], f16, tag=f"pm{q}")
                nc.sync.dma_start(out=pt[:],
                                  in_=pmats_d[:, q * chunk:(q + 1) * chunk])
                pm_t.append(pt)

            # Each exp window must be its OWN psum tensor: Tile serializes
            # PE-writes vs ACT-reads within one PSUM tensor regardless of
            # bank. Alternate a 4-bank and a 3-bank tile (4+3+1(R)=8 banks).
            R = psr.tile([103, 512], f32, tag="R")              # 1 bank

            windows = []
            g = 0
            m = 0
            while g < NG:
                n = min(4 if m % 2 == 0 else 3, NG - g)
                windows.append((m % 2, n, g))
                g += n
                m += 1

            loop_cm = (tc.For_i(0, loop_n, 1,
                                hint_engines=(mybir.EngineType.PE,
                                              mybir.EngineType.Activation),
                                staggered_reset=stag)
                       if loop_n else contextlib.nullcontext())
            with loop_cm:
                for rep in range(repeat if not loop_n else 1):
                    pending = []

                    def emit_pv(ent):
                        n, g0, we = ent
                        for k in range(n):
                            gg = g0 + k
                            i, j = gg // NS, gg % NS
                            nc.tensor.matmul(
                                R[32 * j:32 * j + 7, :],
                                pc_t[:, 7 * i:7 * i + 7],
                                we[:, 512 * k:512 * (k + 1)],
                                start=(i == 0), stop=(i == NT - 1),
                                skip_group_check=True,
                                tile_position=(0, 32 * j))

                    for (par, n, g0) in windows:
                        wt = psw.tile([128, 2048 if par == 0 else 1536],
                                      f32, tag=f"w{par}")
                        for k in range(n):
                            gg = g0 + k
                            i, j = gg // NS, gg % NS
                            lhs = pm_t[i // (NT // 4)]
                            ci = (i % (NT // 4)) * 128
                            nc.tensor.matmul(
                                wt[:, 512 * k:512 * (k + 1)],
                                lhs[:, ci:ci + 128],
                                xmat_t[:, 512 * j:512 * (j + 1)],
                                start=True, stop=True)
                        we = wpool.tile([128, 2048 if par == 0 else 1536],
                                        f16, tag=f"we{par}")
                        nc.scalar.activation(
                            we[:, 0:512 * n], wt[:, 0:512 * n],
                            mybir.ActivationFunctionType.Exp)
                        pending.append((n, g0, we))
                        if len(pending) > skew_w:
                            emit_pv(pending.pop(0))
                    for ent in pending:
                        emit_pv(ent)
            r_sb = const.tile([128, 512], f32, tag="r_sb")
            for j in range(NS):
                nc.vector.tensor_copy(r_sb[32 * j:32 * j + 7, :],
                                      R[32 * j:32 * j + 7, :])
                nc.sync.dma_start(out=rout_d[7 * j:7 * j + 7, :],
                                  in_=r_sb[32 * j:32 * j + 7, :])
    nc.compile()
    return nc


def _get_program(S, P_core, repeat=1, fuse_ldw=False, loop_n=None,
                 skew=0, we_bufs=3):
    key = (S, P_core, repeat, fuse_ldw, loop_n, skew, we_bufs)
    if key not in _PROGRAM_CACHE:
        _PROGRAM_CACHE[key] = _build_program(S, P_core, repeat, fuse_ldw,
                                             loop_n, skew, we_bufs)
    return _PROGRAM_CACHE[key]


def _get_program_best(S, P_core, loop_n=None):
    key = ("best", S, P_core, loop_n)
    if key not in _PROGRAM_CACHE:
        _PROGRAM_CACHE[key] = _build_program_v3(S, P_core, loop_n=loop_n,
                                                skew_w=2, we_bufs=3)
    return _PROGRAM_CACHE[key]


def _make_in_maps(d):
    P_core = d["P"] // N_CORES
    NT = P_core // 128
    in_maps = []
    for c in range(N_CORES):
        sl = slice(c * P_core, (c + 1) * P_core)
        pc_block = d["pc_aug"][sl].reshape(NT, 128, 7)
        pc_core = np.ascontiguousarray(
            pc_block.transpose(1, 0, 2).reshape(128, NT * 7))
        in_maps.append({
            "xmat": d["xmat_hl"],
            "pmats": np.ascontiguousarray(d["pmat_hl"][:, sl]),
            "pcents": pc_core,
        })
    return in_maps


def _postprocess(d, results):
    S, C, B, H, W = d["S"], d["C"], d["B"], d["H"], d["W"]
    R = np.zeros((28, 512), np.float64)
    for c in range(N_CORES):
        R += results[c]["r_out"].astype(np.float64)
    R = R.reshape(4, 7, 512)
    Rc = (R[:, 0:3, :] + R[:, 3:6, :]).transpose(1, 0, 2).reshape(C, S)
    sw = R[:, 6, :].reshape(S)
    xs = d["x"].transpose(1, 0, 2, 3).reshape(C, S)
    out = (d["mu_t"] * Rc / sw - xs) / d["s2"]
    return np.ascontiguousarray(
        out.reshape(C, B, H, W).transpose(1, 0, 2, 3)).astype(np.float32)


def kernel(x, images, mu, sigma, t):
    from concourse.bass_utils import run_bass_kernel_spmd

    d = _preprocess(x, images, mu, sigma, t)
    assert d["P"] % (N_CORES * 128) == 0
    nc = _get_program_best(d["S"], d["P"] // N_CORES)
    res = run_bass_kernel_spmd(nc, _make_in_maps(d), list(range(N_CORES)))
    return _postprocess(d, res.results)
